# revision 1
# baseline (speedup 1.0000x reference)
"""Differentiable particle filter V3 — Trainium2 Bass kernel.

Strategy: data-parallel over batch B=16 across 8 NeuronCores (2 batch items
per core).  Each core runs the full T=16 sequential scan for its two
particle clouds (N=512, dL=128) with all activations kept on-chip in a
"transposed" (feature-on-partition, particle-on-free) layout so every MLP
layer is a natural PE matmul chain.  The h_t contribution of each layer-1
matmul is folded into a per-batch bias vector (h is constant across the
particle axis).  The NxN Gumbel soft-resample uses the identity
exp(g/T) = x^-2 (T=0.5, x = -log(u+1e-10)+1e-10): the host ships
P^T = x^-2 (transposed, bf16) and the device folds the per-particle
log-weight scale exp(2*(lw - max)) into it, then computes the row-stochastic
mix plus its denominator with two matmul groups (z block, [rlog | ones]
block) contracted over the particle axis.
"""

import math
import numpy as np
import ml_dtypes

import concourse.bass as bass
import concourse.tile as tile
from concourse import mybir
from concourse.masks import make_identity
from concourse.bass import ds, ts

F32 = mybir.dt.float32
F16 = mybir.dt.float16
BF16 = mybir.dt.bfloat16
AF = mybir.ActivationFunctionType
ALU = mybir.AluOpType
AX = mybir.AxisListType

LOG2PI = 1.8378770664093453
TEMP = 0.5

# problem dims (hardcoded per spec)
B, N, T_FULL = 16, 512, 16
dL, dM, dE, H = 128, 256, 32, 256
Kt, Ka = 18, 8
NCORES = 8
B2 = 2          # batch items per core
NC = 4          # 128-particle chunks per batch item
NCB = NC * B2   # total particle chunks per core (8)
NT = B2 * N     # total particles per core (1024)


def split_waits(nc, limit=1):
    """This walrus build encodes at most one sync wait per instruction.
    Hoist excess waits onto injected same-engine NOPs placed immediately
    before the instruction (engine program order preserves semantics)."""
    for f in nc.m.functions:
        for bb in f.blocks:
            newl = []
            for ins in bb.instructions:
                si = ins.sync_info
                if si is not None and si.on_wait and len(si.on_wait) > limit:
                    waits = list(si.on_wait)
                    for k, wv in enumerate(waits[:-limit]):
                        nop = mybir.InstNoOp(
                            name=f"{ins.name}-ws{k}", ins=[], outs=[])
                        nop.engine = ins.engine
                        nop.sync_info = mybir.SyncInfo(on_wait=[wv], on_update=[])
                        newl.append(nop)
                    si.on_wait = waits[-limit:]
                newl.append(ins)
            try:
                bb.instructions = newl
            except Exception:
                bb.instructions.clear()
                bb.instructions.extend(newl)
    return nc


def build_core_program(t_steps=T_FULL):
    """Build the Bass program one core runs (2 batch items, t_steps steps)."""
    nc = bass.Bass()

    # ---------------- DRAM parameters (per-core shapes) ----------------
    d_pT = nc.declare_dram_parameter("pT", [t_steps, B2, N, N], BF16, isOutput=False)
    d_eps = nc.declare_dram_parameter("eps_n", [t_steps, B2, N, dL], F32, isOutput=False)
    d_hT = nc.declare_dram_parameter("hT", [t_steps, dM, B2], F16, isOutput=False)
    d_obs = nc.declare_dram_parameter("obs_b", [t_steps, B2], F32, isOutput=False)
    d_z0T = nc.declare_dram_parameter("z0T", [dL, NT], F16, isOutput=False)
    d_rl0T = nc.declare_dram_parameter("rl0T", [Kt, NT], F32, isOutput=False)

    # weights (fp16 for PE), biases / misc consts (fp32)
    d_w = {}
    for name, shape, dt in [
        ("pz1_z", [dL, H], F16), ("pz1_e", [dE, H], F16), ("pz1_h", [dM, H], F16),
        ("pr1_z", [dL, H], F16), ("pr1_e", [dE, H], F16), ("pr1_h", [dM, H], F16),
        ("pz2", [H, H], F16), ("pz3", [H, 2 * dL], F16),
        ("pr2", [H, Ka], F16),
        ("oe1_z", [dL, H], F16), ("oe1_h", [dM, H], F16),
        ("oe2", [H, 128], F16), ("oe3", [128, 2], F16),
        ("emb8", [Ka, dE], F16),
        ("b1z", [128, 2], F32),    # pz_b1 as two 128-col tiles
        ("b1r", [128, 2], F32),    # pr_b1
        ("b2z", [128, 2], F32),    # pz_b2
        ("b1o", [128, 2], F32),    # oe_b1
        ("b2o", [128, 1], F32),    # oe_b2
        ("b3m_t", [128, dL], F32),   # pz_b3[:128] replicated rows
        ("b3s_t", [128, dL], F32),   # pz_b3[128:] replicated rows
        ("prb2_t", [128, Ka], F32),  # pr_b2 replicated rows
        ("oe3b_t", [128, 2], F32),   # oe_b3 replicated rows
        ("scales_t", [128, Ka], F32),  # softplus(log_obs_scale[:8]) replicated
    ]:
        d_w[name] = nc.declare_dram_parameter(name, shape, dt, isOutput=False)

    d_out = nc.declare_dram_parameter("means", [B2, dL, t_steps], F32, isOutput=True)

    from contextlib import ExitStack
    with tile.TileContext(nc) as tc, ExitStack() as ctx:
        wp = ctx.enter_context(tc.tile_pool(name="wp", bufs=1))
        sp = ctx.enter_context(tc.tile_pool(name="sp", bufs=2))
        st = ctx.enter_context(tc.tile_pool(name="st", bufs=1))
        ps = ctx.enter_context(tc.tile_pool(name="ps", bufs=6, space="PSUM"))
        dp = ctx.enter_context(tc.tile_pool(name="dp", bufs=2, space="DRAM"))

        def psum(shape, name, dt=F32):
            return ps.tile(shape, dt, tag="ps", name=name)

        # ---------------- load weights / constants ----------------
        w = {}
        for name, h in d_w.items():
            shp = list(h.shape)
            if shp[0] > 128:
                assert shp[0] == 256
                tl = wp.tile([128, 2, shp[1]], h.dtype, name="w_" + name)
                nc.gpsimd.dma_start(
                    out=tl, in_=h[:, :].rearrange("(c p) m -> p c m", p=128))
            else:
                tl = wp.tile(shp, h.dtype, name="w_" + name)
                nc.gpsimd.dma_start(out=tl, in_=h[:, :])
            w[name] = tl

        ident16 = wp.tile([128, 128], F16, name="ident16")
        make_identity(nc, ident16)
        ident32 = wp.tile([128, 128], F32, name="ident32")
        make_identity(nc, ident32)
        ones18_f32 = wp.tile([Kt, 1], F32, name="ones18")
        nc.vector.memset(ones18_f32, 1.0)
        ones_row = wp.tile([1, 128], F32, name="ones_row")
        nc.vector.memset(ones_row, 1.0)
        ones_bf = wp.tile([128, 1], BF16, name="ones_bf")
        nc.vector.memset(ones_bf, 1.0)

        def bcast128(src11, name):
            """(1,1) fp32 -> (128,1) SBUF via ones-matmul broadcast."""
            pb = psum([128, 1], "bc")
            nc.tensor.matmul(pb, ones_row, src11[0:1, 0:1], start=True, stop=True)
            out = sp.tile([128, 1], F32, name=name)
            nc.vector.tensor_copy(out=out, in_=pb)
            return out

        # ---------------- state ----------------
        zT = st.tile([dL, NT], F16, name="zT_state")          # z^T carry (f16)
        nc.gpsimd.dma_start(out=zT, in_=d_z0T[:, :])
        rlr_rows = st.tile([Ka, NT], F32, name="rlr_rows")    # rl carry rows (t>=1)
        means_acc = st.tile([dL, B2, t_steps], F32, name="means_acc")
        # Collapse the weight/state-load DMA deps: a chain of tiny DVE
        # reads (one per loaded tile, program-ordered on DVE) accumulates
        # every DMA tick into DVE's vector clock; a single NOP barrier
        # depending on the last read then covers all loads, so no later
        # instruction exceeds the per-instruction sync-wait limit.
        from concourse.tile import add_dep_helper
        probe = st.tile([1, 1], F32, name="probe")
        last_copy = None
        for tl in [*w.values(), ident16, ident32, zT]:
            src = tl[0:1, 0, 0:1] if len(tl.shape) == 3 else tl[0:1, 0:1]
            last_copy = nc.vector.tensor_copy(out=probe, in_=src)
        curr_bb = nc.cur_bb
        bar = nc.sync.nop()
        assert last_copy is not None
        add_dep_helper(bar.ins, last_copy.ins, sync=True,
                       reason="weights barrier")
        tc.barrier_instruction_and_bb = (bar.ins, curr_bb)

        for t in range(t_steps):
            # ---------------- step inputs ----------------
            eps = sp.tile([128, NCB, dL], F32, name="eps")   # (p, chunk, d)
            for b in range(B2):
                for jc in range(NC):
                    nc.gpsimd.dma_start(
                        out=eps[:, b * NC + jc, :],
                        in_=d_eps[t, b, ts(jc, 128), :])
            hT_t = sp.tile([128, 2, B2], F16, name="hT_t")   # (p, kchunk, b)
            nc.gpsimd.dma_start(
                out=hT_t, in_=d_hT[t].rearrange("(c p) b -> p c b", p=128))

            # y broadcast to a column per b
            ybc = sp.tile([128, B2], F32, name="ybc")
            for b in range(B2):
                nc.gpsimd.dma_start(
                    out=ybc[:, b:b + 1],
                    in_=d_obs[t:t + 1, b:b + 1].to_broadcast([128, 1]))

            # ---------------- regime softmax -> rp rows (8, NT) f16 -------
            rp_rows = sp.tile([Ka, NT], F16, name="rp_rows")
            if t == 0:
                rl0 = sp.tile([Kt, NT], F32, name="rl0")
                nc.gpsimd.dma_start(out=rl0, in_=d_rl0T[:, :])
                erow0 = sp.tile([Kt, NT], F32, name="erow0")
                nc.scalar.activation(out=erow0, in_=rl0, func=AF.Exp)
                # denominator over all 18 regimes (fp32 matmul, per 512 cols)
                rd0 = sp.tile([1, NT], F32, name="rd0")
                for b in range(B2):
                    dps = psum([1, N], "d18")
                    nc.tensor.matmul(dps, ones18_f32, erow0[:, ts(b, N)],
                                     start=True, stop=True)
                    # 1/D via exp(-ln(D))
                    lnd = sp.tile([1, N], F32, name="lnd0")
                    nc.scalar.activation(out=lnd, in_=dps, func=AF.Ln)
                    nc.scalar.activation(out=rd0[:, ts(b, N)], in_=lnd,
                                         func=AF.Exp, scale=-1.0)
                # broadcast 1/D to 18 partitions via DRAM bounce
                bounce0 = dp.tile([1, NT], F32, name="bounce0")
                nc.gpsimd.dma_start(out=bounce0, in_=rd0)
                rdbc0 = sp.tile([Kt, NT], F32, name="rdbc0")
                nc.gpsimd.dma_start(out=rdbc0,
                                    in_=bounce0[:, :].to_broadcast([Kt, NT]))
                rp0 = sp.tile([Kt, NT], F32, name="rp0")
                nc.vector.tensor_mul(rp0, erow0, rdbc0)
                nc.vector.tensor_copy(out=rp_rows, in_=rp0[0:Ka, :])
            else:
                # rlr_rows (8, NT) f32 -> columns per chunk, softmax, back
                rl_cols = sp.tile([128, NCB, Ka], F32, name="rl_cols")
                for c in range(NCB):
                    tps = psum([128, Ka], "rltp")
                    nc.tensor.transpose(tps, rlr_rows[:, ts(c, 128)],
                                        ident32[0:Ka, 0:Ka])
                    nc.vector.tensor_copy(out=rl_cols[:, c, :], in_=tps)
                ecol = sp.tile([128, NCB, Ka], F32, name="ecol")
                nc.scalar.activation(out=ecol, in_=rl_cols, func=AF.Exp)
                dsum = sp.tile([128, NCB], F32, name="dsum")
                nc.vector.tensor_reduce(out=dsum, in_=ecol, axis=AX.X, op=ALU.add)
                nc.vector.tensor_scalar_add(dsum, dsum, float(Kt - Ka))
                rdc = sp.tile([128, NCB], F32, name="rdc")
                nc.vector.reciprocal(out=rdc, in_=dsum)
                rp_cols = sp.tile([128, NCB, Ka], F32, name="rp_cols")
                for c in range(NCB):
                    nc.vector.tensor_scalar_mul(
                        rp_cols[:, c, :], ecol[:, c, :], rdc[:, c:c + 1])
                for c in range(NCB):
                    tps = psum([Ka, 128], "rptp")
                    nc.tensor.transpose(tps, rp_cols[:, c, :], ident32)
                    nc.vector.tensor_copy(out=rp_rows[:, ts(c, 128)], in_=tps)

            # ---------------- regime embedding (32, NT) f16 ----------------
            rembT = sp.tile([dE, NT], F16, name="rembT")
            for b in range(B2):
                rps = psum([dE, N], "remb")
                nc.tensor.matmul(rps, w["emb8"], rp_rows[:, ts(b, N)],
                                 start=True, stop=True)
                nc.vector.tensor_copy(out=rembT[:, ts(b, N)], in_=rps)

            # ---------------- h-contribution biases (tiny matmuls) --------
            # bias_tiles[name][m] : (128, B2) f32 = W_h[:,m].T @ h + b[m]
            def h_bias(w_h, b_tile, nm):
                out_t = sp.tile([128, 2, B2], F32, name="hb_" + nm)
                for m in range(2):
                    hb = psum([128, B2], "hb")
                    for kc in range(2):
                        nc.tensor.matmul(
                            hb, w_h[:, kc, ts(m, 128)], hT_t[:, kc, :],
                            start=(kc == 0), stop=(kc == 1))
                    nc.vector.tensor_scalar_add(
                        out_t[:, m, :], hb, b_tile[:, m:m + 1])
                return out_t

            bias_z1 = h_bias(w["pz1_h"], w["b1z"], "z1")
            bias_r1 = h_bias(w["pr1_h"], w["b1r"], "r1")
            bias_o1 = h_bias(w["oe1_h"], w["b1o"], "o1")

            # ---------------- proposal layer 1 + 2 ----------------
            def layer1(wz, we, bias_t, nm):
                out = []
                for m in range(2):
                    ht = sp.tile([128, NT], F16, name=nm + str(m))
                    for b in range(B2):
                        ps1 = psum([128, N], "l1")
                        nc.tensor.matmul(ps1, wz[:, ts(m, 128)],
                                         zT[:, ts(b, N)], start=True, stop=False)
                        nc.tensor.matmul(ps1, we[:, ts(m, 128)],
                                         rembT[:, ts(b, N)], start=False, stop=True)
                        nc.scalar.activation(out=ht[:, ts(b, N)], in_=ps1,
                                             func=AF.Silu,
                                             bias=bias_t[:, m, b:b + 1])
                    out.append(ht)
                return out

            hz1 = layer1(w["pz1_z"], w["pz1_e"], bias_z1, "hz1_")
            prh = layer1(w["pr1_z"], w["pr1_e"], bias_r1, "prh_")

            hz2 = []
            for m in range(2):
                ht = sp.tile([128, NT], F16, name="hz2_" + str(m))
                for b in range(B2):
                    ps2 = psum([128, N], "l2")
                    nc.tensor.matmul(ps2, w["pz2"][:, 0, ts(m, 128)],
                                     hz1[0][:, ts(b, N)], start=True, stop=False)
                    nc.tensor.matmul(ps2, w["pz2"][:, 1, ts(m, 128)],
                                     hz1[1][:, ts(b, N)], start=False, stop=True)
                    nc.scalar.activation(out=ht[:, ts(b, N)], in_=ps2,
                                         func=AF.Silu, bias=w["b2z"][:, m:m + 1])
                hz2.append(ht)

            # ---------------- proposal out (flip): zp columns ----------------
            zm_g = sp.tile([128, NT], F32, name="zm_g")     # z_mean cols gathered
            ls_g = sp.tile([128, NT], F32, name="ls_g")     # log_std cols gathered
            for c in range(NCB):
                zps = psum([128, 2 * dL], "zp")
                nc.tensor.matmul(zps, hz2[0][:, ts(c, 128)], w["pz3"][:, 0, :],
                                 start=True, stop=False)
                nc.tensor.matmul(zps, hz2[1][:, ts(c, 128)], w["pz3"][:, 1, :],
                                 start=False, stop=True)
                nc.vector.tensor_add(zm_g[:, ts(c, dL)], zps[:, 0:dL], w["b3m_t"])
                nc.vector.tensor_add(ls_g[:, ts(c, dL)], zps[:, dL:2 * dL],
                                     w["b3s_t"])
            # clip log_std to [-5, 2]
            nc.vector.tensor_scalar(ls_g, ls_g, 2.0, -5.0, op0=ALU.min, op1=ALU.max)
            els = sp.tile([128, NT], F32, name="els")
            nc.scalar.activation(out=els, in_=ls_g, func=AF.Exp)

            # z_new (columns) + log_q
            znew = sp.tile([128, NT], F32, name="znew")
            qsum = sp.tile([128, NCB], F32, name="qsum")
            for c in range(NCB):
                ec = eps[:, c, :]
                nc.vector.tensor_mul(znew[:, ts(c, dL)], ec, els[:, ts(c, dL)])
                nc.vector.tensor_add(znew[:, ts(c, dL)], znew[:, ts(c, dL)],
                                     zm_g[:, ts(c, dL)])
                e2 = sp.tile([128, dL], F32, name="e2")
                nc.vector.tensor_mul(e2, ec, ec)
                qtmp = sp.tile([128, dL], F32, name="qtmp")
                nc.vector.scalar_tensor_tensor(
                    out=qtmp, in0=e2, scalar=0.5,
                    in1=ls_g[:, ts(c, dL)], op0=ALU.mult, op1=ALU.add,
                    accum_out=qsum[:, c:c + 1])
            # log_q = -qsum - dL/2*LOG2PI  (folded later)

            znew16 = sp.tile([128, NT], F16, name="znew16")
            nc.vector.tensor_copy(out=znew16, in_=znew)
            znew_bf = sp.tile([128, NT], BF16, name="znew_bf")
            nc.vector.tensor_copy(out=znew_bf, in_=znew)

            # ---------------- observation net ----------------
            znT = sp.tile([dL, NT], F16, name="znT")
            for c in range(NCB):
                tps = psum([128, 128], "ztp", F16)
                nc.tensor.transpose(tps, znew16[:, ts(c, 128)], ident16)
                nc.vector.tensor_copy(out=znT[:, ts(c, 128)], in_=tps)

            oeh = []
            for m in range(2):
                ht = sp.tile([128, NT], F16, name="oeh_" + str(m))
                for b in range(B2):
                    pso = psum([128, N], "o1")
                    nc.tensor.matmul(pso, w["oe1_z"][:, ts(m, 128)],
                                     znT[:, ts(b, N)], start=True, stop=True)
                    nc.scalar.activation(out=ht[:, ts(b, N)], in_=pso,
                                         func=AF.Silu, bias=bias_o1[:, m, b:b + 1])
                oeh.append(ht)
            em2 = sp.tile([128, NT], F16, name="em2")
            for b in range(B2):
                pso = psum([128, N], "o2")
                nc.tensor.matmul(pso, w["oe2"][:, 0, :], oeh[0][:, ts(b, N)],
                                 start=True, stop=False)
                nc.tensor.matmul(pso, w["oe2"][:, 1, :], oeh[1][:, ts(b, N)],
                                 start=False, stop=True)
                nc.scalar.activation(out=em2[:, ts(b, N)], in_=pso,
                                     func=AF.Silu, bias=w["b2o"][:, 0:1])
            em_c = sp.tile([128, NCB, 2], F32, name="em_c")
            for c in range(NCB):
                pse = psum([128, 2], "o3")
                nc.tensor.matmul(pse, em2[:, ts(c, 128)], w["oe3"],
                                 start=True, stop=True)
                nc.vector.tensor_add(em_c[:, c, :], pse, w["oe3b_t"])
            pred = em_c[:, :, 0]   # (128, NCB) strided views
            lsb = em_c[:, :, 1]

            # ---------------- pr layer-2 (flip): rlog columns --------------
            rlog_c = sp.tile([128, NCB, Ka], F32, name="rlog_c")
            for c in range(NCB):
                psr = psum([128, Ka], "pr2")
                nc.tensor.matmul(psr, prh[0][:, ts(c, 128)], w["pr2"][:, 0, :],
                                 start=True, stop=False)
                nc.tensor.matmul(psr, prh[1][:, ts(c, 128)], w["pr2"][:, 1, :],
                                 start=False, stop=True)
                nc.vector.tensor_add(rlog_c[:, c, :], psr, w["prb2_t"])

            # rp_new softmax (over 18 = 8 + 10 zeros) in columns
            ern = sp.tile([128, NCB, Ka], F32, name="ern")
            nc.scalar.activation(out=ern, in_=rlog_c, func=AF.Exp)
            dn = sp.tile([128, NCB], F32, name="dn")
            nc.vector.tensor_reduce(out=dn, in_=ern, axis=AX.X, op=ALU.add)
            nc.vector.tensor_scalar_add(dn, dn, float(Kt - Ka))
            rdn = sp.tile([128, NCB], F32, name="rdn")
            nc.vector.reciprocal(out=rdn, in_=dn)
            # sigma multiplier: sum_k rp_new * scales  (fold 1/D once at end)
            smu = sp.tile([128, NCB, Ka], F32, name="smu")
            for c in range(NCB):
                nc.vector.tensor_mul(smu[:, c, :], ern[:, c, :], w["scales_t"])
            smult = sp.tile([128, NCB], F32, name="smult")
            nc.vector.tensor_reduce(out=smult, in_=smu, axis=AX.X, op=ALU.add)
            nc.vector.tensor_mul(smult, smult, rdn)

            # ---------------- sigma, log-likelihood, weights ----------------
            sp_t = sp.tile([128, NCB], F32, name="sp_t")
            nc.scalar.activation(out=sp_t, in_=lsb, func=AF.Exp)
            nc.scalar.activation(out=sp_t, in_=sp_t, func=AF.Ln, bias=1.0)
            sig = sp.tile([128, NCB], F32, name="sig")
            nc.vector.tensor_mul(sig, sp_t, smult)
            nc.vector.tensor_scalar(sig, sig, 5.0, 0.1, op0=ALU.min, op1=ALU.max)
            rsig = sp.tile([128, NCB], F32, name="rsig")
            nc.vector.reciprocal(out=rsig, in_=sig)
            dev = sp.tile([128, NCB], F32, name="dev")
            for b in range(B2):
                nc.vector.tensor_scalar_sub(
                    dev[:, b * NC:(b + 1) * NC],
                    pred[:, b * NC:(b + 1) * NC], ybc[:, b:b + 1])
            nc.vector.tensor_mul(dev, dev, rsig)
            nc.vector.tensor_mul(dev, dev, dev)       # ((pred-y)/sig)^2
            lns = sp.tile([128, NCB], F32, name="lns")
            nc.scalar.activation(out=lns, in_=sig, func=AF.Ln)
            # lw = -0.5*dev - lns - 0.5*LOG2PI + qsum + 64*LOG2PI
            lw = sp.tile([128, NCB], F32, name="lw")
            nc.vector.scalar_tensor_tensor(
                out=lw, in0=dev, scalar=-0.5, in1=lns,
                op0=ALU.mult, op1=ALU.subtract)
            nc.vector.tensor_add(lw, lw, qsum)
            nc.vector.tensor_scalar_add(lw, lw, (dL - 1) * 0.5 * LOG2PI)

            # per-batch max over particles -> negMb (1,1) -> broadcast
            nmb_bc = sp.tile([128, B2], F32, name="nmb_bc")
            for b in range(B2):
                mx = sp.tile([128, 1], F32, name="mx")
                nc.vector.tensor_reduce(
                    out=mx, in_=lw[:, b * NC:(b + 1) * NC], axis=AX.X, op=ALU.max)
                nmx = sp.tile([1, 1], F32, name="nmx")
                nc.gpsimd.tensor_reduce(out=nmx, in_=mx, axis=AX.C, op=ALU.max)
                nc.gpsimd.tensor_scalar_mul(nmx, nmx, -1.0)
                nmbb = bcast128(nmx, "nmbb")
                nc.vector.tensor_copy(out=nmb_bc[:, b:b + 1], in_=nmbb)

            # e_w = exp(lw - Mb);  s_j = exp(2*(lw - Mb))
            e_w = sp.tile([128, NCB], F32, name="e_w")
            s_j = sp.tile([128, NCB], F32, name="s_j")
            for b in range(B2):
                sl = slice(b * NC, (b + 1) * NC)
                nc.scalar.activation(out=e_w[:, sl], in_=lw[:, sl], func=AF.Exp,
                                     bias=nmb_bc[:, b:b + 1])
                cj = sp.tile([128, NC], F32, name="cj")
                nc.vector.tensor_scalar(
                    cj, lw[:, sl], nmb_bc[:, b:b + 1], 2.0,
                    op0=ALU.add, op1=ALU.mult)
                nc.scalar.activation(out=s_j[:, sl], in_=cj, func=AF.Exp)

            # ---------------- weighted mean output ----------------
            for b in range(B2):
                mz = psum([128, 1], "mz")
                for jc in range(NC):
                    c = b * NC + jc
                    nc.tensor.matmul(mz, znew[:, ts(c, dL)], e_w[:, c:c + 1],
                                     start=(jc == 0), stop=(jc == NC - 1))
                sew = sp.tile([128, 1], F32, name="sew")
                nc.vector.tensor_reduce(
                    out=sew, in_=e_w[:, b * NC:(b + 1) * NC], axis=AX.X, op=ALU.add)
                sews = sp.tile([1, 1], F32, name="sews")
                nc.gpsimd.tensor_reduce(out=sews, in_=sew, axis=AX.C, op=ALU.add)
                rse = sp.tile([1, 1], F32, name="rse")
                nc.vector.reciprocal(out=rse, in_=sews)
                rse_bc = bcast128(rse, "rse_bc")
                nc.vector.tensor_scalar_mul(means_acc[:, b, t:t + 1], mz, rse_bc)

            # ---------------- soft resample ----------------
            rl9 = sp.tile([128, NCB, Ka], BF16, name="rl9")
            nc.vector.tensor_copy(out=rl9, in_=rlog_c)

            for b in range(B2):
                g1 = psum([dL, N], "g1")
                g2 = psum([Ka, N], "g2")
                gd = psum([1, N], "gd")
                for jc in range(NC):
                    c = b * NC + jc
                    pch = sp.tile([128, N], BF16, name="pch")
                    nc.gpsimd.dma_start(
                        out=pch,
                        in_=d_pT[t, b].rearrange("(c p) i -> p c i", p=128)[:, jc, :])
                    ech = sp.tile([128, N], BF16, name="ech")
                    nc.vector.tensor_scalar_mul(ech, pch, s_j[:, c:c + 1])
                    nc.tensor.matmul(g1, znew_bf[:, ts(c, dL)], ech,
                                     start=(jc == 0), stop=(jc == NC - 1))
                    nc.tensor.matmul(g2, rl9[:, c, :], ech,
                                     start=(jc == 0), stop=(jc == NC - 1))
                    nc.tensor.matmul(gd, ones_bf, ech,
                                     start=(jc == 0), stop=(jc == NC - 1))
                # 1/D row via exp(-ln(D)), broadcast via DRAM bounce
                rd = sp.tile([1, N], F32, name="rd")
                nc.scalar.activation(out=rd, in_=gd, func=AF.Ln)
                nc.scalar.activation(out=rd, in_=rd, func=AF.Exp, scale=-1.0)
                bounce = dp.tile([1, N], F32, name="bounce")
                nc.gpsimd.dma_start(out=bounce, in_=rd)
                rdbc = sp.tile([128, N], F32, name="rdbc")
                nc.gpsimd.dma_start(out=rdbc,
                                    in_=bounce[:, :].to_broadcast([128, N]))
                # normalized carries
                nc.vector.tensor_mul(zT[:, ts(b, N)], g1, rdbc[0:dL, :])
                nc.vector.tensor_mul(rlr_rows[:, ts(b, N)], g2,
                                     rdbc[0:Ka, :])

        # ---------------- write outputs ----------------
        for b in range(B2):
            nc.gpsimd.dma_start(out=d_out[b], in_=means_acc[:, b, :])

    return split_waits(nc)


# ======================= host side =======================

def _f16(x):
    return np.asarray(x, dtype=np.float32).astype(ml_dtypes.float16 if False else np.float16)


def _bf16(x):
    return np.asarray(x, dtype=np.float32).astype(ml_dtypes.bfloat16)


def _rep(row, p=128):
    return np.broadcast_to(np.asarray(row, np.float32)[None, :], (p, len(row))).copy()


def host_prep(inputs, t_steps=T_FULL):
    """Build the 8 per-core input maps."""
    obs = np.asarray(inputs["obs"], np.float32)[:t_steps]
    h_seq = np.asarray(inputs["h_seq"], np.float32)[:t_steps]
    z0 = np.asarray(inputs["z0"], np.float32)
    rl0 = np.asarray(inputs["regime_logits0"], np.float32)
    eps = np.asarray(inputs["eps"], np.float32)[:t_steps]
    u = np.asarray(inputs["gumbel_u"], np.float32)[:t_steps]
    assert int(inputs["k_active"]) == Ka

    # exp(g/TEMP) = x^-2  with x = -log(u+1e-10)+1e-10 (TEMP=0.5)
    x = (-np.log(u + np.float32(1e-10)) + np.float32(1e-10)).astype(np.float32)
    P = (1.0 / (x * x)).astype(np.float32)

    wmap = {
        "pz1_z": _f16(inputs["pz_w1"][dM:dM + dL]),
        "pz1_e": _f16(inputs["pz_w1"][dM + dL:]),
        "pz1_h": _f16(inputs["pz_w1"][:dM]),
        "pr1_z": _f16(inputs["pr_w1"][dM:dM + dL]),
        "pr1_e": _f16(inputs["pr_w1"][dM + dL:]),
        "pr1_h": _f16(inputs["pr_w1"][:dM]),
        "pz2": _f16(inputs["pz_w2"]), "pz3": _f16(inputs["pz_w3"]),
        "pr2": _f16(inputs["pr_w2"]),
        "oe1_z": _f16(inputs["oe_w1"][:dL]),
        "oe1_h": _f16(inputs["oe_w1"][dL:]),
        "oe2": _f16(inputs["oe_w2"]), "oe3": _f16(inputs["oe_w3"]),
        "emb8": _f16(inputs["pe_emb"][:Ka]),
        "b1z": np.asarray(inputs["pz_b1"], np.float32).reshape(2, 128).T.copy(),
        "b1r": np.asarray(inputs["pr_b1"], np.float32).reshape(2, 128).T.copy(),
        "b2z": np.asarray(inputs["pz_b2"], np.float32).reshape(2, 128).T.copy(),
        "b1o": np.asarray(inputs["oe_b1"], np.float32).reshape(2, 128).T.copy(),
        "b2o": np.asarray(inputs["oe_b2"], np.float32).reshape(1, 128).T.copy(),
        "b3m_t": _rep(np.asarray(inputs["pz_b3"], np.float32)[:dL]),
        "b3s_t": _rep(np.asarray(inputs["pz_b3"], np.float32)[dL:]),
        "prb2_t": _rep(np.asarray(inputs["pr_b2"], np.float32)),
        "oe3b_t": _rep(np.asarray(inputs["oe_b3"], np.float32)),
        "scales_t": _rep(np.log1p(np.exp(
            np.asarray(inputs["log_obs_scale"], np.float32)[:Ka]))),
    }

    in_maps = []
    for core in range(NCORES):
        bp = [2 * core, 2 * core + 1]
        m = dict(wmap)
        m["pT"] = _bf16(P[:, bp].transpose(0, 1, 3, 2))          # (T,2,j,i)
        m["eps_n"] = eps[:, bp]                                   # (T,2,N,dL)
        m["hT"] = _f16(h_seq[:, bp].transpose(0, 2, 1))           # (T,dM,2)
        m["obs_b"] = obs[:, bp]                                   # (T,2)
        m["z0T"] = _f16(np.concatenate(
            [z0[b].T for b in bp], axis=1))                       # (dL, 1024)
        m["rl0T"] = np.concatenate(
            [rl0[b].T for b in bp], axis=1).astype(np.float32)    # (18, 1024)
        in_maps.append(m)
    return in_maps


def gather_output(results, t_steps=T_FULL):
    out = np.zeros((t_steps, B, dL), np.float32)
    for core in range(NCORES):
        r = results[core]["means"]                                # (2,128,T)
        for b in range(B2):
            out[:, 2 * core + b, :] = np.asarray(r[b], np.float32).T
    return out


def kernel(**inputs):
    from concourse.bass_utils import run_bass_kernel_spmd
    nc = build_core_program(T_FULL)
    in_maps = host_prep(inputs, T_FULL)
    res = run_bass_kernel_spmd(nc, in_maps, list(range(NCORES)))
    return gather_output(res.results, T_FULL)


if __name__ == "__main__":
    nc = build_core_program(2)
    print("built ok")

